# revision 1
# baseline (speedup 1.0000x reference)
"""Trainium2 Bass kernel for a dense pre-norm transformer block (v2).

Sharding: sequence-parallel over 8 cores (512 tokens each; cores 0-3 own
batch 0, cores 4-7 own batch 1). k/v are AllGathered (bf16) within each
4-core batch group; everything else is local. Host gather is concatenation.

Platform notes (measured): DMA bandwidth is the bottleneck (~3GB/s/core with
8 cores active), so weights ship as bf16 and are converted on-chip to fp32r
(bf16 matmuls are slow here due to LDWEIGHTS; fp32r self-loads). All matmuls
are fp32r with N=512. PSUM tiles are [128,1024] (2 banks) to halve
instruction counts.
"""

import numpy as np
import ml_dtypes

import concourse.bass as bass
import concourse.mybir as mybir
import concourse.tile as tile
import bass_rust
from concourse.bass import ts
from concourse.bass_utils import run_bass_kernel_spmd

B, N, C = 2, 2048, 1024
H, DH = 16, 64
HID = 4096
EPS = 1e-6
N_CORES = 8
T = (B * N) // N_CORES          # 512 tokens per core
TT = T // 128                   # 4
CC = C // 128                   # 8
FT = HID // 128                 # 32
KT = N // 128                   # 16
NPAIR = H // 2                  # 8

FP32 = mybir.dt.float32
FP32R = mybir.dt.float32r
BF16 = mybir.dt.bfloat16
AF = mybir.ActivationFunctionType
ALU = mybir.AluOpType
BF = ml_dtypes.bfloat16


def _split_multiwait(nc):
    """starfish walrus supports only one sync-wait per instruction; hoist
    extras onto preceding nops on the same engine."""
    counter = 0
    for fn in nc.m.functions:
        for bb in fn.blocks:
            changed = False
            new_insts = []
            for inst in bb.instructions:
                si = inst.sync_info
                if si is not None and len(si.on_wait) > 1:
                    changed = True
                    waits = list(si.on_wait)
                    for w in waits[:-1]:
                        counter += 1
                        nop = bass_rust.InstNoOp(name=f"waitsplit-{counter}")
                        nop.engine = inst.engine
                        nop.sync_info = bass_rust.SyncInfo(on_wait=[w], on_update=[])
                        new_insts.append(nop)
                    inst.sync_info = bass_rust.SyncInfo(
                        on_wait=[waits[-1]], on_update=list(si.on_update)
                    )
                new_insts.append(inst)
            if changed:
                bb.instructions = new_insts
    return counter


def build_nc(mock_gather=False):
    nc = bass.Bass(num_devices=N_CORES)

    x_d = nc.dram_tensor("x", [T, C], FP32, kind="ExternalInput")
    wqk_d = nc.dram_tensor("wqk", [16, 128, 1024], BF16, kind="ExternalInput")
    wv_d = nc.dram_tensor("wv", [8, 128, 1024], BF16, kind="ExternalInput")
    wproj_d = nc.dram_tensor("wproj", [8, 128, 1024], BF16, kind="ExternalInput")
    wmlp1_d = nc.dram_tensor("wmlp1", [32, 128, 1024], BF16, kind="ExternalInput")
    wmlp2_d = nc.dram_tensor("wmlp2", [32, 128, 1024], BF16, kind="ExternalInput")
    bqk_d = nc.dram_tensor("bqk", [128, 16], FP32, kind="ExternalInput")
    bv_d = nc.dram_tensor("bv", [1, C], FP32R, kind="ExternalInput")
    bproj_d = nc.dram_tensor("bproj", [1, C], FP32R, kind="ExternalInput")
    b1_d = nc.dram_tensor("b1", [128, FT], FP32, kind="ExternalInput")
    bmlp2_d = nc.dram_tensor("bmlp2", [1, C], FP32R, kind="ExternalInput")
    ident_d = nc.dram_tensor("ident", [128, 128], FP32, kind="ExternalInput")
    ones_d = nc.dram_tensor("ones", [1, 128], FP32R, kind="ExternalInput")
    onescol_d = nc.dram_tensor("onescol", [128, H], FP32R, kind="ExternalInput")
    out_d = nc.dram_tensor("out", [T, C], FP32, kind="ExternalOutput")

    tensors = dict(
        x_d=x_d, wqk_d=wqk_d, wv_d=wv_d, wproj_d=wproj_d, wmlp1_d=wmlp1_d,
        wmlp2_d=wmlp2_d, bqk_d=bqk_d, bv_d=bv_d, bproj_d=bproj_d, b1_d=b1_d,
        bmlp2_d=bmlp2_d, ident_d=ident_d, ones_d=ones_d, onescol_d=onescol_d,
        out_d=out_d,
    )
    with tile.TileContext(nc) as tc:
        _body(nc, tc, tensors, mock_gather)
    nsplit = _split_multiwait(nc)
    return nc, nsplit


def _body(nc, tc, d, mock_gather):
    from contextlib import ExitStack

    x_d = d["x_d"]; wqk_d = d["wqk_d"]; wv_d = d["wv_d"]
    wproj_d = d["wproj_d"]; wmlp1_d = d["wmlp1_d"]; wmlp2_d = d["wmlp2_d"]
    bqk_d = d["bqk_d"]; bv_d = d["bv_d"]; bproj_d = d["bproj_d"]
    b1_d = d["b1_d"]; bmlp2_d = d["bmlp2_d"]; ident_d = d["ident_d"]
    ones_d = d["ones_d"]; onescol_d = d["onescol_d"]; out_d = d["out_d"]

    ctx = ExitStack()
    with ctx:
        consts = ctx.enter_context(tc.tile_pool(name="consts", bufs=1))
        xpool = ctx.enter_context(tc.tile_pool(name="xpool", bufs=1))
        actp = ctx.enter_context(tc.tile_pool(name="actp", bufs=1))
        kpool = ctx.enter_context(tc.tile_pool(name="kpool", bufs=1))
        ppool = ctx.enter_context(tc.tile_pool(name="ppool", bufs=2))
        wpool = ctx.enter_context(tc.tile_pool(name="wpool", bufs=2))
        stg = ctx.enter_context(tc.tile_pool(name="stg", bufs=2))
        misc = ctx.enter_context(tc.tile_pool(name="misc", bufs=2))
        psum = ctx.enter_context(tc.tile_pool(name="psum", bufs=4, space="PSUM"))
        dram = ctx.enter_context(tc.tile_pool(name="dram", bufs=1, space="DRAM"))

        # ---- constants ----
        ident = consts.tile([128, 128], FP32)
        nc.sync.dma_start(out=ident[:], in_=ident_d[:])
        ones = consts.tile([1, 128], FP32R)
        nc.sync.dma_start(out=ones[:], in_=ones_d[:])
        onescol = consts.tile([128, H], FP32R)
        nc.sync.dma_start(out=onescol[:], in_=onescol_d[:])
        bqk = consts.tile([128, 16], FP32)
        nc.sync.dma_start(out=bqk[:], in_=bqk_d[:])
        bv = consts.tile([1, C], FP32R)
        nc.sync.dma_start(out=bv[:], in_=bv_d[:])
        bproj = consts.tile([1, C], FP32R)
        nc.sync.dma_start(out=bproj[:], in_=bproj_d[:])
        b1c = consts.tile([128, FT], FP32)
        nc.sync.dma_start(out=b1c[:], in_=b1_d[:])
        bmlp2 = consts.tile([1, C], FP32R)
        nc.sync.dma_start(out=bmlp2[:], in_=bmlp2_d[:])
        eps_t = consts.tile([128, 1], FP32)
        nc.vector.memset(eps_t[:], EPS)

        k_loc = dram.tile([C, T], BF16, tag="k_loc")
        k_gath = dram.tile([4 * C, T], BF16, tag="k_gath")
        v_loc = dram.tile([T, C], BF16, tag="v_loc")
        v_gath = dram.tile([N, C], BF16, tag="v_gath")
        rg = [[0, 1, 2, 3], [4, 5, 6, 7]]

        def gather(src, dst):
            if mock_gather:
                nblk = dst.shape[0] // src.shape[0]
                for r in range(nblk):
                    nc.sync.dma_start(
                        out=dst[r * src.shape[0] : (r + 1) * src.shape[0], :],
                        in_=src[:],
                    )
            else:
                nc.gpsimd.collective_compute(
                    "AllGather", ALU.bypass, replica_groups=rg,
                    ins=[src[:].opt()], outs=[dst[:].opt()],
                )

        def loadconv(dram_t, idx, name):
            wbf = wpool.tile([128, 1024], BF16, tag="wbf", bufs=4, name=f"wbf_{name}")
            nc.sync.dma_start(out=wbf[:], in_=dram_t[idx, :, :])
            wfp = wpool.tile([128, 1024], FP32R, tag="wfp", bufs=3, name=f"wfp_{name}")
            nc.scalar.activation(out=wfp[:], in_=wbf[:], func=AF.Copy, scale=1.0)
            return wfp

        def ln_transpose(x_ap, dst, tt, nm):
            """token-major [128, C] tile -> normalized transpose into
            dst[:, :, tt*128:...] (fp32r)."""
            stats = misc.tile([128, 2, 6], FP32, tag="bnstats", name=f"bs{nm}")
            xr = x_ap.rearrange("p (s f) -> p s f", s=2)
            for s in range(2):
                nc.vector.bn_stats(out=stats[:, s, :], in_=xr[:, s, :])
            mv = misc.tile([128, 2], FP32, tag="bnmv", name=f"mv{nm}")
            nc.vector.bn_aggr(out=mv[:], in_=stats[:])
            rstd = misc.tile([128, 1], FP32, tag="rstd", name=f"rs{nm}")
            nc.scalar.activation(
                out=rstd[:], in_=mv[:, 1:2], func=AF.Sqrt, bias=eps_t[:], scale=1.0
            )
            nc.vector.reciprocal(out=rstd[:], in_=rstd[:])
            negmr = misc.tile([128, 1], FP32, tag="negmr", name=f"nm{nm}")
            nc.vector.tensor_scalar(
                out=negmr[:], in0=mv[:, 0:1], scalar1=rstd[:], scalar2=-1.0,
                op0=ALU.mult, op1=ALU.mult,
            )
            xh = stg.tile([128, C], FP32, tag="s4k", name=f"xh{nm}")
            nc.scalar.activation(
                out=xh[:], in_=x_ap, func=AF.Identity, scale=rstd[:], bias=negmr[:]
            )
            pt = psum.tile([128, 1024], FP32, tag="ps", name=f"tp{nm}")
            for cc in range(CC):
                nc.tensor.transpose(
                    pt[:, ts(cc, 128)], xh[:, ts(cc, 128)], ident[:]
                )
            nc.scalar.activation(
                out=dst[:, :, ts(tt, 128)],
                in_=pt[:].rearrange("p (c t) -> p c t", c=CC),
                func=AF.Copy, scale=1.0,
            )

        # ================ Phase A: load x, LN1, transpose ================
        x_sb = xpool.tile([128, TT, C], FP32, tag="xsb")
        nc.sync.dma_start(
            out=x_sb[:], in_=x_d[:].rearrange("(tt p) c -> p tt c", p=128)
        )
        xnT = actp.tile([128, CC, T], FP32R, tag="t16", bufs=2)
        for tt in range(TT):
            ln_transpose(x_sb[:, tt, :], xnT, tt, f"a{tt}")

        # ================ Phase B: QKV ================
        qT = actp.tile([128, NPAIR, T], FP32R, tag="t16", bufs=2)
        for half in range(2):
            accs = []
            for j in range(4):
                acc = psum.tile([128, 1024], FP32, tag="ps", name=f"qk{half}{j}")
                accs.append(acc)
            for cc in range(CC):
                wfp = loadconv(wqk_d, half * 8 + cc, f"qk{half}{cc}")
                for j in range(4):
                    nc.tensor.matmul(
                        accs[j][:, 0:512], wfp[:, ts(2 * j, 128)], xnT[:, cc, :],
                        start=(cc == 0), stop=(cc == CC - 1),
                    )
                    nc.tensor.matmul(
                        accs[j][:, 512:1024], wfp[:, ts(2 * j + 1, 128)],
                        xnT[:, cc, :],
                        start=(cc == 0), stop=(cc == CC - 1),
                    )
            for j in range(4):
                for b in range(2):
                    mt = half * 8 + 2 * j + b
                    blk = accs[j][:, b * 512 : (b + 1) * 512]
                    if half == 0:
                        nc.scalar.activation(
                            out=qT[:, 2 * j + b, :], in_=blk, func=AF.Identity,
                            scale=1.0, bias=bqk[:, mt : mt + 1],
                        )
                    else:
                        kst = stg.tile([128, 512], BF16, tag="sbf",
                                       name=f"kst{j}{b}")
                        nc.scalar.activation(
                            out=kst[:], in_=blk, func=AF.Identity,
                            scale=1.0, bias=bqk[:, mt : mt + 1],
                        )
                        nc.sync.dma_start(
                            out=k_loc[ts(2 * j + b, 128), :], in_=kst[:]
                        )
        gather(k_loc, k_gath)

        # v (token-major)
        vaccs = []
        for tt in range(TT):
            vacc = psum.tile([128, 1024], FP32, tag="ps", name=f"v{tt}")
            vaccs.append(vacc)
        for cc in range(CC):
            wfp = loadconv(wv_d, cc, f"v{cc}")
            for tt in range(TT):
                for hb in range(2):
                    nc.tensor.matmul(
                        vaccs[tt][:, hb * 512 : (hb + 1) * 512],
                        xnT[:, cc, ts(tt, 128)],
                        wfp[:, hb * 512 : (hb + 1) * 512],
                        start=(cc == 0), stop=False,
                    )
        for tt in range(TT):
            for hb in range(2):
                nc.tensor.matmul(
                    vaccs[tt][:, hb * 512 : (hb + 1) * 512],
                    ones[:, 0:128], bv[:, hb * 512 : (hb + 1) * 512],
                    start=False, stop=True,
                )
            vst = stg.tile([128, C], BF16, tag="sbf", name=f"vst{tt}")
            nc.scalar.activation(out=vst[:], in_=vaccs[tt][:], func=AF.Copy,
                                 scale=1.0)
            nc.sync.dma_start(out=v_loc[ts(tt, 128), :], in_=vst[:])
        gather(v_loc, v_gath)

        # vaug: [128 ktok, kt, h, 65] fp32r = [v | 1]
        vaug = actp.tile([128, KT, H, 65], FP32R, tag="t66")
        for kt in range(KT):
            vb = stg.tile([128, C], BF16, tag="sbf", name=f"vb{kt}")
            nc.sync.dma_start(out=vb[:], in_=v_gath[ts(kt, 128), :])
            nc.scalar.activation(
                out=vaug[:, kt, :, 0:64],
                in_=vb[:].rearrange("p (h d) -> p h d", h=H),
                func=AF.Copy, scale=1.0,
            )
            nc.scalar.activation(
                out=vaug[:, kt, :, 64:65],
                in_=onescol[:].rearrange("p (h o) -> p h o", o=1),
                func=AF.Copy, scale=1.0,
            )

        # ================ Phase C: attention ================
        yT = actp.tile([128, NPAIR, T], FP32R, tag="yT")
        for p in range(NPAIR):
            kp = kpool.tile([128, N], FP32R, tag="kp", bufs=1, name=f"kp{p}")
            for rank in range(4):
                kbf = kpool.tile([128, 512], BF16, tag="kbf", bufs=2,
                                 name=f"kbf{p}_{rank}")
                nc.sync.dma_start(
                    out=kbf[:],
                    in_=k_gath[rank * C + p * 128 : rank * C + (p + 1) * 128, :],
                )
                nc.scalar.activation(out=kp[:, ts(rank, 512)], in_=kbf[:],
                                     func=AF.Copy, scale=1.0)

            y = psum.tile([128, 1024], FP32, tag="ps", name=f"y{p}")
            for kt in range(KT):
                sc = psum.tile([128, 1024], FP32, tag="ps", name=f"sc{p}_{kt}")
                nc.tensor.matmul(
                    sc[:, 0:512], kp[0:64, ts(kt, 128)], qT[0:64, p, :],
                    start=True, stop=True, tile_position=(0, 0),
                )
                nc.tensor.matmul(
                    sc[:, 512:1024], kp[64:128, ts(kt, 128)], qT[64:128, p, :],
                    start=True, stop=True, tile_position=(64, 0),
                )
                pt = ppool.tile([128, 1024], FP32R, tag="pt", name=f"pt{p}_{kt}")
                nc.scalar.activation(out=pt[:], in_=sc[:], func=AF.Exp, scale=1.0)
                nc.tensor.matmul(
                    y[0:65, 0:512], vaug[:, kt, 2 * p, :], pt[:, 0:512],
                    start=(kt == 0), stop=(kt == KT - 1),
                )
                nc.tensor.matmul(
                    y[0:65, 512:1024], vaug[:, kt, 2 * p + 1, :], pt[:, 512:1024],
                    start=(kt == 0), stop=(kt == KT - 1),
                )
            rec = misc.tile([1, 1024], FP32R, tag="rec", bufs=1, name=f"rec{p}")
            with nc.allow_low_precision(reason="softmax 1/S scale"):
                nc.vector.reciprocal(out=rec[0:1, 0:512], in_=y[64:65, 0:512])
                nc.vector.reciprocal(
                    out=rec[0:1, 512:1024], in_=y[64:65, 512:1024]
                )
            rb = psum.tile([128, 1024], FP32, tag="ps", name=f"rb{p}")
            nc.tensor.matmul(rb[0:64, 0:512], ones[:, 0:64], rec[0:1, 0:512],
                             start=True, stop=True)
            nc.tensor.matmul(rb[0:64, 512:1024], ones[:, 0:64],
                             rec[0:1, 512:1024], start=True, stop=True)
            rbs = stg.tile([64, 1024], FP32, tag="s4k", name=f"rbs{p}")
            nc.scalar.activation(out=rbs[:], in_=rb[0:64, :], func=AF.Copy,
                                 scale=1.0)
            nc.vector.tensor_tensor(
                out=yT[0:64, p, :], in0=y[0:64, 0:512], in1=rbs[:, 0:512],
                op=ALU.mult,
            )
            nc.vector.tensor_tensor(
                out=yT[64:128, p, :], in0=y[0:64, 512:1024],
                in1=rbs[:, 512:1024], op=ALU.mult,
            )

        # ================ Phase D: proj + residual + LN2 + transpose ========
        x2 = actp.tile([128, TT, C], FP32, tag="t16", bufs=2)
        paccs = []
        for tt in range(TT):
            pacc = psum.tile([128, 1024], FP32, tag="ps", name=f"pj{tt}")
            paccs.append(pacc)
        for dc in range(NPAIR):
            wfp = loadconv(wproj_d, dc, f"pj{dc}")
            for tt in range(TT):
                for hb in range(2):
                    nc.tensor.matmul(
                        paccs[tt][:, hb * 512 : (hb + 1) * 512],
                        yT[:, dc, ts(tt, 128)],
                        wfp[:, hb * 512 : (hb + 1) * 512],
                        start=(dc == 0), stop=False,
                    )
        for tt in range(TT):
            for hb in range(2):
                nc.tensor.matmul(
                    paccs[tt][:, hb * 512 : (hb + 1) * 512],
                    ones[:, 0:128], bproj[:, hb * 512 : (hb + 1) * 512],
                    start=False, stop=True,
                )
            nc.vector.tensor_tensor(
                out=x2[:, tt, :], in0=x_sb[:, tt, :], in1=paccs[tt][:],
                op=ALU.add,
            )

        x2hT = actp.tile([128, CC, T], FP32R, tag="t16", bufs=2)
        for tt in range(TT):
            ln_transpose(x2[:, tt, :], x2hT, tt, f"d{tt}")

        # ================ Phase E: MLP ================
        hT = actp.tile([128, FT, T], FP32R, tag="t66")
        for fg in range(4):
            haccs = []
            for j in range(4):
                hacc = psum.tile([128, 1024], FP32, tag="ps", name=f"h{fg}{j}")
                haccs.append(hacc)
            for cc in range(CC):
                wfp = loadconv(wmlp1_d, fg * 8 + cc, f"m1_{fg}{cc}")
                for j in range(4):
                    nc.tensor.matmul(
                        haccs[j][:, 0:512], wfp[:, ts(2 * j, 128)],
                        x2hT[:, cc, :],
                        start=(cc == 0), stop=(cc == CC - 1),
                    )
                    nc.tensor.matmul(
                        haccs[j][:, 512:1024], wfp[:, ts(2 * j + 1, 128)],
                        x2hT[:, cc, :],
                        start=(cc == 0), stop=(cc == CC - 1),
                    )
            for j in range(4):
                for b in range(2):
                    ft = fg * 8 + 2 * j + b
                    nc.scalar.activation(
                        out=hT[:, ft, :],
                        in_=haccs[j][:, b * 512 : (b + 1) * 512],
                        func=AF.Gelu, scale=1.0, bias=b1c[:, ft : ft + 1],
                    )

        oaccs = []
        for tt in range(TT):
            oacc = psum.tile([128, 1024], FP32, tag="ps", name=f"o{tt}")
            oaccs.append(oacc)
        for fc in range(FT):
            wfp = loadconv(wmlp2_d, fc, f"m2_{fc}")
            for tt in range(TT):
                for hb in range(2):
                    nc.tensor.matmul(
                        oaccs[tt][:, hb * 512 : (hb + 1) * 512],
                        hT[:, fc, ts(tt, 128)],
                        wfp[:, hb * 512 : (hb + 1) * 512],
                        start=(fc == 0), stop=False,
                    )
        for tt in range(TT):
            for hb in range(2):
                nc.tensor.matmul(
                    oaccs[tt][:, hb * 512 : (hb + 1) * 512],
                    ones[:, 0:128], bmlp2[:, hb * 512 : (hb + 1) * 512],
                    start=False, stop=True,
                )
            ot = stg.tile([128, C], FP32, tag="s4k", name=f"ot{tt}")
            nc.vector.tensor_tensor(
                out=ot[:], in0=x2[:, tt, :], in1=oaccs[tt][:], op=ALU.add
            )
            nc.sync.dma_start(out=out_d[ts(tt, 128), :], in_=ot[:])


_NC_CACHE = {}


def _get_nc():
    if "nc" not in _NC_CACHE:
        _NC_CACHE["nc"] = build_nc()[0]
    return _NC_CACHE["nc"]


def _host_prep(inputs):
    f32 = np.float32
    x = np.asarray(inputs["x"], f32).reshape(B * N, C)
    ln1_g = np.asarray(inputs["ln1_g"], f32)
    ln1_b = np.asarray(inputs["ln1_b"], f32)
    w_qkv = np.asarray(inputs["w_qkv"], f32)
    w_proj = np.asarray(inputs["w_proj"], f32)
    b_proj = np.asarray(inputs["b_proj"], f32)
    ln2_g = np.asarray(inputs["ln2_g"], f32)
    ln2_b = np.asarray(inputs["ln2_b"], f32)
    w_mlp1 = np.asarray(inputs["w_mlp1"], f32)
    b_mlp1 = np.asarray(inputs["b_mlp1"], f32)
    w_mlp2 = np.asarray(inputs["w_mlp2"], f32)
    b_mlp2 = np.asarray(inputs["b_mlp2"], f32)

    scale = DH ** -0.5
    wqkv_eff = (w_qkv * ln1_g[:, None]).astype(f32).copy()
    wqkv_eff[:, :C] *= scale
    bqkv = (ln1_b @ w_qkv).astype(f32)
    bqkv[:C] *= scale
    bqk = np.ascontiguousarray(bqkv[: 2 * C].reshape(16, 128).T)
    bv = np.ascontiguousarray(bqkv[2 * C :].reshape(1, C))

    # pre-tiled bf16 weight blocks (each [128, 1024] block contiguous)
    wqk = np.empty((16, 128, 1024), BF)
    for half in range(2):
        for cc in range(CC):
            wqk[half * 8 + cc] = wqkv_eff[
                cc * 128 : (cc + 1) * 128, half * 1024 : (half + 1) * 1024
            ].astype(BF)
    wv = np.empty((8, 128, 1024), BF)
    for cc in range(CC):
        wv[cc] = wqkv_eff[cc * 128 : (cc + 1) * 128, 2 * C : 3 * C].astype(BF)
    wproj_t = np.empty((8, 128, 1024), BF)
    for dc in range(8):
        wproj_t[dc] = w_proj[dc * 128 : (dc + 1) * 128, :].astype(BF)
    wmlp1_eff = (w_mlp1 * ln2_g[:, None]).astype(f32)
    wmlp1_t = np.empty((32, 128, 1024), BF)
    for fg in range(4):
        for cc in range(CC):
            wmlp1_t[fg * 8 + cc] = wmlp1_eff[
                cc * 128 : (cc + 1) * 128, fg * 1024 : (fg + 1) * 1024
            ].astype(BF)
    wmlp2_t = np.empty((32, 128, 1024), BF)
    for fc in range(32):
        wmlp2_t[fc] = w_mlp2[fc * 128 : (fc + 1) * 128, :].astype(BF)

    b1_eff = (b_mlp1 + ln2_b @ w_mlp1).astype(f32)
    b1 = np.ascontiguousarray(b1_eff.reshape(FT, 128).T)

    common = {
        "wqk": wqk,
        "wv": wv,
        "wproj": wproj_t,
        "wmlp1": wmlp1_t,
        "wmlp2": wmlp2_t,
        "bqk": bqk,
        "bv": bv,
        "bproj": np.ascontiguousarray(b_proj.reshape(1, C)),
        "b1": b1,
        "bmlp2": np.ascontiguousarray(b_mlp2.reshape(1, C)),
        "ident": np.eye(128, dtype=f32),
        "ones": np.ones((1, 128), f32),
        "onescol": np.ones((128, H), f32),
    }
    in_maps = []
    for c in range(N_CORES):
        m = dict(common)
        m["x"] = np.ascontiguousarray(x[c * T : (c + 1) * T, :])
        in_maps.append(m)
    return in_maps


def kernel(**inputs):
    nc = _get_nc()
    in_maps = _host_prep(inputs)
    res = run_bass_kernel_spmd(nc, in_maps, core_ids=list(range(N_CORES)))
    out = np.concatenate(
        [res.results[c]["out"] for c in range(N_CORES)], axis=0
    )
    return out.reshape(B, N, C).astype(np.float32)



# revision 2
# speedup vs baseline: 1.0283x; 1.0283x over previous
"""Trainium2 Bass kernel for a dense pre-norm transformer block (v3: 1-core).

Measured platform behavior (this axon-tunneled fleet):
  - DMA is a shared resource: 1 active core gets ~38-85 GB/s, but 2+ cores
    simultaneously DMAing collapse to ~1.4-4.5 GB/s/core. So the fastest
    design runs the WHOLE block on ONE core: ~59 MB of traffic at solo
    bandwidth beats 8 cores fighting over the pipe.
  - PE: ~0.8-1.2 ns per matmul row; fp8 DoubleRow halves row count.
    ACT: ~96 G elem/s; exp of the 134M attention scores overlaps under PE.

Design: core 0 processes both batches sequentially. Weights ship as
fp8(e4m3, kept <=240 for TRN range) with power-of-2 scales folded into
evac rescales; big GEMMs (QKV, V, PV, proj, MLP1, MLP2) run fp8 DoubleRow
([K,2,N] contraction-pair planes); QK runs fp8 pair-packed via
tile_position. Residual spine bf16, LN stats fp32. Free-dim biases
(bv/bproj) fold into the x reload (host adds bv@w_proj + b_proj); bmlp2
enters via a [1,2,128] ones-plane DR matmul. SBUF: one 16.25KB-slot pool
(9 bufs) rotates all per-batch residents; PSUM: one 2-bank-slot pool
(4 bufs) = all 8 banks.
"""

import numpy as np
import ml_dtypes

import concourse.bass as bass
import concourse.mybir as mybir
import concourse.tile as tile
import bass_rust
from concourse.bass import ts
from concourse.bass_utils import run_bass_kernel_spmd

B, N, C = 2, 2048, 1024
H, DH = 16, 64
HID = 4096
EPS = 1e-6
T = 2048                 # tokens per batch
NTT = 16                 # 128-token tiles per batch
CC = 8                   # feature blocks
NP = 8                   # head pairs
KT = 16                  # k tiles per batch
NHB = 32                 # mlp hidden blocks
NJP = 4                  # c-block pairs (DR planes over 1024 contraction)
NHJP = 16                # h-block pairs (DR planes over 4096 contraction)

FP32 = mybir.dt.float32
FP32R = mybir.dt.float32r
BF16 = mybir.dt.bfloat16
FP8 = mybir.dt.float8e4
AF = mybir.ActivationFunctionType
ALU = mybir.AluOpType
DR = mybir.MatmulPerfMode.DoubleRow
BF = ml_dtypes.bfloat16
F8 = mybir.dt.np(FP8)

# power-of-2 scales; evac rescales are exact
S_X = 4.0       # x_n -> fp8
S_WQ = 1024.0   # wq (incl 1/sqrt(dh)) -> fp8
S_WK = 128.0
S_WV = 128.0
S_Q = 32.0      # qT fp8
S_K = 4.0       # kT fp8
S_V = 8.0       # vaug fp8 (ones col = S_V, denominators cancel)
S_Y = 4.0       # yT fp8 (via ones4 broadcast value)
S_WP = 128.0
S_W1 = 128.0
S_H = 1.0       # hT fp8
S_W2 = 128.0

E_QK = 1.0 / (S_Q * S_K)          # exp() input rescale
R_Q = S_Q / (S_WQ * S_X)          # q psum -> qT
R_K = S_K / (S_WK * S_X)
R_V = S_V / (S_WV * S_X)
R_P = 1.0 / (S_Y * S_WP)          # proj psum -> true attn out
R_1 = 1.0 / (S_W1 * S_X)          # mlp1 psum -> true h (pre-gelu)
R_2 = 1.0 / (S_H * S_W2)


def _split_multiwait(nc):
    """starfish walrus supports only one sync-wait per instruction; hoist
    extras onto preceding nops on the same engine."""
    counter = 0
    for fn in nc.m.functions:
        for bb in fn.blocks:
            changed = False
            new_insts = []
            for inst in bb.instructions:
                si = inst.sync_info
                if si is not None and len(si.on_wait) > 1:
                    changed = True
                    waits = list(si.on_wait)
                    for w in waits[:-1]:
                        counter += 1
                        nop = bass_rust.InstNoOp(name=f"waitsplit-{counter}")
                        nop.engine = inst.engine
                        nop.sync_info = bass_rust.SyncInfo(on_wait=[w], on_update=[])
                        new_insts.append(nop)
                    inst.sync_info = bass_rust.SyncInfo(
                        on_wait=[waits[-1]], on_update=list(si.on_update)
                    )
                new_insts.append(inst)
            if changed:
                bb.instructions = new_insts
    return counter


def build_nc():
    nc = bass.Bass(num_devices=1)

    x_d = nc.dram_tensor("x", [2 * NTT, 128, C], BF16, kind="ExternalInput")
    xc_d = nc.dram_tensor("xc", [2 * NTT, 128, C], BF16, kind="ExternalInput")
    wqk_d = nc.dram_tensor("wqk", [128, 2 * NP * NJP * 2 * 128], FP8,
                           kind="ExternalInput")
    wv_d = nc.dram_tensor("wv", [128, NJP * 2 * C], FP8, kind="ExternalInput")
    wproj_d = nc.dram_tensor("wproj", [128, NJP * 2 * C], FP8,
                             kind="ExternalInput")
    w1_d = nc.dram_tensor("w1", [128, NJP, 2, HID], FP8, kind="ExternalInput")
    w1r_d = nc.dram_tensor("w1r", [128, NJP, 2, HID], FP8, kind="ExternalInput")
    w2_d = nc.dram_tensor("w2", [128, NHJP, 2, C], FP8, kind="ExternalInput")
    w2r_d = nc.dram_tensor("w2r", [128, NHJP, 2, C], FP8, kind="ExternalInput")
    bqk_d = nc.dram_tensor("bqk", [128, 16], FP32, kind="ExternalInput")
    b1_d = nc.dram_tensor("b1", [128, NHB], FP32, kind="ExternalInput")
    e00_d = nc.dram_tensor("e00", [1, 2 * 128], FP8, kind="ExternalInput")
    bm2_d = nc.dram_tensor("bm2", [1, 2 * C], FP8, kind="ExternalInput")
    ident_d = nc.dram_tensor("ident", [128, 128], BF16, kind="ExternalInput")
    ones4_d = nc.dram_tensor("ones4", [1, 64], FP32R, kind="ExternalInput")
    out_d = nc.dram_tensor("out", [2 * NTT, 128, C], FP32, kind="ExternalOutput")

    tensors = dict(
        x_d=x_d, xc_d=xc_d, wqk_d=wqk_d, wv_d=wv_d, wproj_d=wproj_d,
        w1_d=w1_d, w1r_d=w1r_d, w2_d=w2_d, w2r_d=w2r_d, bqk_d=bqk_d,
        b1_d=b1_d, e00_d=e00_d, bm2_d=bm2_d, ident_d=ident_d,
        ones4_d=ones4_d, out_d=out_d,
    )
    with tile.TileContext(nc) as tc:
        _body(nc, tc, tensors)
    nsplit = _split_multiwait(nc)
    return nc, nsplit


def _body(nc, tc, d):
    from contextlib import ExitStack

    ctx = ExitStack()
    with ctx:
        consts = ctx.enter_context(tc.tile_pool(name="consts", bufs=1))
        pF = ctx.enter_context(tc.tile_pool(name="pF", bufs=10))
        stg = ctx.enter_context(tc.tile_pool(name="stg", bufs=2))
        misc = ctx.enter_context(tc.tile_pool(name="misc", bufs=2))
        psum = ctx.enter_context(tc.tile_pool(name="psum", bufs=4, space="PSUM"))
        dram = ctx.enter_context(tc.tile_pool(name="dram", bufs=1, space="DRAM"))

        ident = consts.tile([128, 128], BF16)
        nc.sync.dma_start(out=ident[:], in_=d["ident_d"][:])
        ones4 = consts.tile([1, 64], FP32R)
        nc.sync.dma_start(out=ones4[:], in_=d["ones4_d"][:])
        bqk = consts.tile([128, 16], FP32)
        nc.sync.dma_start(out=bqk[:], in_=d["bqk_d"][:])
        b1c = consts.tile([128, NHB], FP32)
        nc.sync.dma_start(out=b1c[:], in_=d["b1_d"][:])
        e00 = consts.tile([1, 2, 128], FP8)
        nc.sync.dma_start(out=e00[:],
                          in_=d["e00_d"][:].rearrange("o (t e) -> o t e", t=2))
        bm2 = consts.tile([1, 2, C], FP8)
        nc.sync.dma_start(out=bm2[:],
                          in_=d["bm2_d"][:].rearrange("o (t c) -> o t c", t=2))
        eps_t = consts.tile([128, 1], FP32)
        nc.vector.memset(eps_t[:], EPS)

        x2sp = dram.tile([2 * NTT, 128, C], BF16, tag="x2sp")
        cst = dict(ident=ident, ones4=ones4, bqk=bqk, b1c=b1c, e00=e00,
                   bm2=bm2, eps_t=eps_t, x2sp=x2sp)
        for b in range(2):
            _batch(nc, tc, d, b, pF, stg, misc, psum, cst)


def _batch(nc, tc, d, b, pF, stg, misc, psum, cst):
    x_d = d["x_d"]; xc_d = d["xc_d"]; out_d = d["out_d"]
    ident = cst["ident"]; ones4 = cst["ones4"]; bqk = cst["bqk"]
    b1c = cst["b1c"]; e00 = cst["e00"]; bm2 = cst["bm2"]; eps_t = cst["eps_t"]
    x2sp = cst["x2sp"]

    # ---- per-batch fp8 weight residents (one 16.25KB-slot pool) ----
    wqk = pF.tile([128, 2, NP, NJP, 2, 128], FP8, tag="big", name=f"wqk{b}")
    nc.sync.dma_start(out=wqk[:], in_=d["wqk_d"][:].rearrange(
        "p (q h j t e) -> p q h j t e", q=2, h=NP, j=NJP, t=2))
    wv = pF.tile([128, NJP, 2, C], FP8, tag="big", name=f"wv{b}")
    nc.sync.dma_start(out=wv[:], in_=d["wv_d"][:].rearrange(
        "p (j t c) -> p j t c", j=NJP, t=2))

    qT = pF.tile([128, NP, T], FP8, tag="big", name=f"qT{b}")
    kT = pF.tile([128, NP, T], FP8, tag="big", name=f"kT{b}")
    vaug = pF.tile([128, KT, H, 65], FP8, tag="big", name=f"vaug{b}")
    nc.vector.memset(vaug[:, :, :, 64:65], S_V)

    def ln_tile(xf, nm, persist=False):
        """[128, C] -> (rstdS, negmrS) with S_X folded."""
        stats = misc.tile([128, 2, 6], FP32, tag="bnstats", name=f"bs{nm}")
        xr = xf.rearrange("p (s f) -> p s f", s=2)
        for s in range(2):
            nc.vector.bn_stats(out=stats[:, s, :], in_=xr[:, s, :])
        mv = misc.tile([128, 2], FP32, tag="bnmv", name=f"mv{nm}")
        nc.vector.bn_aggr(out=mv[:], in_=stats[:])
        rstd = misc.tile([128, 1], FP32, tag="rstd", name=f"rs{nm}")
        nc.scalar.activation(
            out=rstd[:], in_=mv[:, 1:2], func=AF.Sqrt, bias=eps_t[:], scale=1.0
        )
        nc.vector.reciprocal(out=rstd[:], in_=rstd[:])
        kw = dict(tag="lnP", bufs=2 * NTT + 2) if persist else dict(tag="rstdS")
        rstdS = misc.tile([128, 1], FP32, name=f"rS{nm}", **kw)
        nc.vector.tensor_scalar(
            out=rstdS[:], in0=rstd[:], scalar1=S_X, scalar2=None, op0=ALU.mult
        )
        negmrS = misc.tile([128, 1], FP32, name=f"nm{nm}", **kw)
        nc.vector.tensor_scalar(
            out=negmrS[:], in0=mv[:, 0:1], scalar1=rstdS[:], scalar2=-1.0,
            op0=ALU.mult, op1=ALU.mult,
        )
        return rstdS, negmrS

    # ================ A+B fused: LN1 -> xnc -> QKV (per 512-tok chunk) ====
    for ch in range(4):
        xnc = stg.tile([128, CC, 512], FP8, tag="xnc", bufs=1, name=f"xnc{b}{ch}")
        for tt in range(4):
            gt = b * NTT + ch * 4 + tt
            xf = stg.tile([128, C], BF16, tag="xf", bufs=2, name=f"xf{b}{ch}{tt}")
            nc.sync.dma_start(out=xf[:], in_=x_d[gt, :, :])
            rstdS, negmrS = ln_tile(xf[:], f"a{b}{ch}{tt}")
            xh = stg.tile([128, C], BF16, tag="xh", bufs=1, name=f"xh{b}{ch}{tt}")
            nc.scalar.activation(
                out=xh[:], in_=xf[:], func=AF.Identity, scale=rstdS[:],
                bias=negmrS[:],
            )
            tp = psum.tile([128, 1024], BF16, tag="ps", name=f"tpA{b}{ch}{tt}")
            for cc in range(CC):
                nc.tensor.transpose(tp[:, ts(cc, 128)], xh[:, ts(cc, 128)],
                                    ident[:])
            nc.vector.tensor_copy(
                out=xnc[:, :, ts(tt, 128)],
                in_=tp[:].rearrange("p (c t) -> p c t", c=CC),
            )
        xncj = xnc[:].rearrange("p (j2 t) f -> p j2 t f", j2=NJP)
        for p in range(NP):
            for qk in range(2):
                ps = psum.tile([128, 512], FP32, tag="ps", name=f"qk{b}{ch}{p}{qk}")
                for j in range(NJP):
                    nc.tensor.matmul(
                        ps[:], wqk[:, qk, p, j, :, :], xncj[:, j, :, :],
                        start=(j == 0), stop=(j == NJP - 1), perf_mode=DR,
                    )
                dst = qT if qk == 0 else kT
                nc.vector.tensor_scalar(
                    out=dst[:, p, ts(ch, 512)], in0=ps[:],
                    scalar1=(R_Q if qk == 0 else R_K),
                    scalar2=bqk[:, qk * 8 + p : qk * 8 + p + 1],
                    op0=ALU.mult, op1=ALU.add,
                )
        for tt in range(4):
            kt = ch * 4 + tt
            pv = psum.tile([128, 1024], FP32, tag="ps", name=f"v{b}{ch}{tt}")
            for j in range(NJP):
                for hh in range(2):
                    nc.tensor.matmul(
                        pv[:, ts(hh, 512)],
                        xnc[:, 2 * j : 2 * j + 2, ts(tt, 128)],
                        wv[:, j, :, ts(hh, 512)],
                        start=(j == 0), stop=(j == NJP - 1), perf_mode=DR,
                    )
            nc.vector.tensor_scalar(
                out=vaug[:, kt, :, 0:64],
                in0=pv[:].rearrange("p (h e) -> p h e", h=H),
                scalar1=R_V, scalar2=None, op0=ALU.mult,
            )

    # ================ C: attention ================
    yT = pF.tile([128, NP, T], FP8, tag="big", name=f"yT{b}")
    for p in range(NP):
        for qc in range(4):
            y = psum.tile([128, 1024], FP32, tag="ps", name=f"y{b}{p}{qc}")
            for ktp in range(8):
                pt = stg.tile([128, 2, 1024], FP8, tag="pt", bufs=2,
                              name=f"pt{b}{p}{qc}{ktp}")
                for par in range(2):
                    kt = 2 * ktp + par
                    sc = psum.tile([128, 1024], FP32, tag="ps",
                                   name=f"sc{b}{p}{qc}{kt}")
                    nc.tensor.matmul(
                        sc[:, 0:512], kT[0:64, p, ts(kt, 128)],
                        qT[0:64, p, ts(qc, 512)],
                        start=True, stop=True, tile_position=(0, 0),
                    )
                    nc.tensor.matmul(
                        sc[:, 512:1024], kT[64:128, p, ts(kt, 128)],
                        qT[64:128, p, ts(qc, 512)],
                        start=True, stop=True, tile_position=(64, 0),
                    )
                    nc.scalar.activation(out=pt[:, par, :], in_=sc[:],
                                         func=AF.Exp, scale=E_QK)
                nc.tensor.matmul(
                    y[0:65, 0:512], vaug[:, 2 * ktp : 2 * ktp + 2, 2 * p, :],
                    pt[:, :, 0:512],
                    start=(ktp == 0), stop=(ktp == 7), perf_mode=DR,
                )
                nc.tensor.matmul(
                    y[0:65, 512:1024],
                    vaug[:, 2 * ktp : 2 * ktp + 2, 2 * p + 1, :],
                    pt[:, :, 512:1024],
                    start=(ktp == 0), stop=(ktp == 7), perf_mode=DR,
                )
            rec = misc.tile([1, 1024], FP32R, tag="rec", bufs=1, name=f"rec{b}{p}{qc}")
            with nc.allow_low_precision(reason="softmax 1/S scale"):
                nc.vector.reciprocal(out=rec[0:1, 0:512], in_=y[64:65, 0:512])
                nc.vector.reciprocal(out=rec[0:1, 512:1024],
                                     in_=y[64:65, 512:1024])
            rb = psum.tile([64, 1024], FP32, tag="ps", name=f"rb{b}{p}{qc}")
            nc.tensor.matmul(rb[:, 0:512], ones4[:], rec[0:1, 0:512],
                             start=True, stop=True)
            nc.tensor.matmul(rb[:, 512:1024], ones4[:], rec[0:1, 512:1024],
                             start=True, stop=True)
            rbs = stg.tile([64, 1024], FP32, tag="rbs", bufs=1,
                           name=f"rbs{b}{p}{qc}")
            nc.scalar.activation(out=rbs[:], in_=rb[:], func=AF.Copy, scale=1.0)
            nc.vector.tensor_tensor(
                out=yT[0:64, p, ts(qc, 512)], in0=y[0:64, 0:512],
                in1=rbs[:, 0:512], op=ALU.mult,
            )
            nc.vector.tensor_tensor(
                out=yT[64:128, p, ts(qc, 512)], in0=y[0:64, 512:1024],
                in1=rbs[:, 512:1024], op=ALU.mult,
            )

    # ================ D: proj + residual + LN2 stats ================
    wp = pF.tile([128, NJP, 2, C], FP8, tag="big", name=f"wp{b}")
    nc.sync.dma_start(out=wp[:], in_=d["wproj_d"][:].rearrange(
        "p (j t c) -> p j t c", j=NJP, t=2))
    lnstats = []
    for tt in range(NTT):
        pp = psum.tile([128, 1024], FP32, tag="ps", name=f"pj{b}{tt}")
        for j in range(NJP):
            for hh in range(2):
                nc.tensor.matmul(
                    pp[:, ts(hh, 512)],
                    yT[:, 2 * j : 2 * j + 2, ts(tt, 128)],
                    wp[:, j, :, ts(hh, 512)],
                    start=(j == 0), stop=(j == NJP - 1), perf_mode=DR,
                )
        xcf = stg.tile([128, C], BF16, tag="xf", bufs=2, name=f"xc{b}{tt}")
        nc.sync.dma_start(out=xcf[:], in_=xc_d[b * NTT + tt, :, :])
        tmp = stg.tile([128, C], BF16, tag="tmp", bufs=1, name=f"tj{b}{tt}")
        nc.vector.tensor_scalar(out=tmp[:], in0=pp[:], scalar1=R_P,
                                scalar2=None, op0=ALU.mult)
        x2t = stg.tile([128, C], BF16, tag="x2t", bufs=2, name=f"x2t{b}{tt}")
        nc.vector.tensor_tensor(out=x2t[:], in0=xcf[:], in1=tmp[:], op=ALU.add)
        nc.sync.dma_start(out=x2sp[b * NTT + tt, :, :], in_=x2t[:])
        lnstats.append(ln_tile(x2t[:], f"d{b}{tt}", persist=True))

    # ================ E: LN2-transpose + MLP ================
    x2hT = pF.tile([128, CC, T], FP8, tag="big", name=f"x2hT{b}")
    rx2hT = pF.tile([128, CC, T], FP8, tag="big", name=f"rx2hT{b}")
    for tt in range(NTT):
        rstdS, negmrS = lnstats[tt]
        xr = stg.tile([128, C], BF16, tag="x2t", bufs=2, name=f"xr{b}{tt}")
        nc.sync.dma_start(out=xr[:], in_=x2sp[b * NTT + tt, :, :])
        x2h = stg.tile([128, C], BF16, tag="xh", bufs=1, name=f"x2h{b}{tt}")
        nc.scalar.activation(out=x2h[:], in_=xr[:],
                             func=AF.Identity, scale=rstdS[:], bias=negmrS[:])
        tp = psum.tile([128, 1024], BF16, tag="ps", name=f"tpE{b}{tt}")
        for cc in range(CC):
            nc.tensor.transpose(tp[:, ts(cc, 128)], x2h[:, ts(cc, 128)],
                                ident[:])
        tpv = tp[:].rearrange("p (c t) -> p c t", c=CC)
        nc.vector.tensor_copy(out=x2hT[:, :, ts(tt, 128)], in_=tpv)
        nc.vector.tensor_tensor(out=rx2hT[:, :, ts(tt, 128)], in0=tpv,
                                in1=x2hT[:, :, ts(tt, 128)], op=ALU.subtract)

    w2_ = [
        pF.tile([128, NHJP // 2, 2, C], FP8, tag="big", name=f"w2a{b}"),
        pF.tile([128, NHJP // 2, 2, C], FP8, tag="big", name=f"w2b{b}"),
    ]
    w2r_ = [
        pF.tile([128, NHJP // 2, 2, C], FP8, tag="big", name=f"w2ra{b}"),
        pF.tile([128, NHJP // 2, 2, C], FP8, tag="big", name=f"w2rb{b}"),
    ]
    for hf in range(2):
        nc.sync.dma_start(out=w2_[hf][:],
                          in_=d["w2_d"][:, ts(hf, NHJP // 2), :, :])
        nc.sync.dma_start(out=w2r_[hf][:],
                          in_=d["w2r_d"][:, ts(hf, NHJP // 2), :, :])

    hT_ = [pF.tile([128, NHB, 512], FP8, tag="big", name=f"hT{b}{qc}")
           for qc in range(4)]
    for hb in range(NHB):
        w1t = stg.tile([128, NJP, 2, 128], FP8, tag="w1s", bufs=3,
                       name=f"w1{b}{hb}")
        nc.sync.dma_start(out=w1t[:], in_=d["w1_d"][:, :, :, ts(hb, 128)])
        w1rt = stg.tile([128, NJP, 2, 128], FP8, tag="w1rs", bufs=3,
                        name=f"w1r{b}{hb}")
        nc.sync.dma_start(out=w1rt[:], in_=d["w1r_d"][:, :, :, ts(hb, 128)])
        for qc in range(4):
            ps = psum.tile([128, 512], FP32, tag="ps", name=f"h{b}{hb}{qc}")
            for j in range(NJP):
                nc.tensor.matmul(
                    ps[:], w1t[:, j, :, :],
                    x2hT[:, 2 * j : 2 * j + 2, ts(qc, 512)],
                    start=(j == 0), stop=False, perf_mode=DR,
                )
            for j in range(NJP):
                nc.tensor.matmul(
                    ps[:], w1t[:, j, :, :],
                    rx2hT[:, 2 * j : 2 * j + 2, ts(qc, 512)],
                    start=False, stop=False, perf_mode=DR,
                )
            for j in range(NJP):
                nc.tensor.matmul(
                    ps[:], w1rt[:, j, :, :],
                    x2hT[:, 2 * j : 2 * j + 2, ts(qc, 512)],
                    start=False, stop=(j == NJP - 1), perf_mode=DR,
                )
            nc.scalar.activation(
                out=hT_[qc][:, hb, :], in_=ps[:], func=AF.Gelu,
                scale=R_1, bias=b1c[:, hb : hb + 1],
            )
    for tt in range(NTT):
        hsrc = hT_[tt // 4]
        tt4 = tt % 4
        po = psum.tile([128, 1024], FP32, tag="ps", name=f"o{b}{tt}")
        for j in range(NHJP):
            for hh in range(2):
                nc.tensor.matmul(
                    po[:, ts(hh, 512)],
                    hsrc[:, 2 * j : 2 * j + 2, ts(tt4, 128)],
                    w2_[j // 8][:, j % 8, :, ts(hh, 512)],
                    start=(j == 0), stop=False, perf_mode=DR,
                )
        for j in range(NHJP):
            for hh in range(2):
                nc.tensor.matmul(
                    po[:, ts(hh, 512)],
                    hsrc[:, 2 * j : 2 * j + 2, ts(tt4, 128)],
                    w2r_[j // 8][:, j % 8, :, ts(hh, 512)],
                    start=False, stop=False, perf_mode=DR,
                )
        for hh in range(2):
            nc.tensor.matmul(
                po[:, ts(hh, 512)], e00[:], bm2[:, :, ts(hh, 512)],
                start=False, stop=True, perf_mode=DR,
            )
        tmp = stg.tile([128, C], BF16, tag="tmp", bufs=1, name=f"to{b}{tt}")
        nc.vector.tensor_scalar(out=tmp[:], in0=po[:], scalar1=R_2,
                                scalar2=None, op0=ALU.mult)
        xr2 = stg.tile([128, C], BF16, tag="x2t", bufs=2, name=f"xr2{b}{tt}")
        nc.sync.dma_start(out=xr2[:], in_=x2sp[b * NTT + tt, :, :])
        ot = stg.tile([128, C], FP32, tag="ot", bufs=1, name=f"ot{b}{tt}")
        nc.vector.tensor_tensor(out=ot[:], in0=xr2[:], in1=tmp[:], op=ALU.add)
        nc.sync.dma_start(out=out_d[b * NTT + tt, :, :], in_=ot[:])


_NC_CACHE = {}


def _get_nc():
    if "nc" not in _NC_CACHE:
        _NC_CACHE["nc"] = build_nc()[0]
    return _NC_CACHE["nc"]


def _pack_pairs(w, s):
    """w [256*nj, M] fp32 -> [128, nj, 2, M] fp8, rows (2j+t)*128 + r."""
    K2, M = w.shape
    nj = K2 // 256
    out = np.empty((128, nj, 2, M), dtype=F8)
    ws = (w * s).astype(np.float32)
    np.clip(ws, -240.0, 240.0, out=ws)
    for j in range(nj):
        for t in range(2):
            out[:, j, t, :] = ws[(2 * j + t) * 128 : (2 * j + t + 1) * 128, :].astype(F8)
    return out


def _host_prep(inputs):
    f32 = np.float32
    x = np.asarray(inputs["x"], f32).reshape(B * N, C)
    ln1_g = np.asarray(inputs["ln1_g"], f32)
    ln1_b = np.asarray(inputs["ln1_b"], f32)
    w_qkv = np.asarray(inputs["w_qkv"], f32)
    w_proj = np.asarray(inputs["w_proj"], f32)
    b_proj = np.asarray(inputs["b_proj"], f32)
    ln2_g = np.asarray(inputs["ln2_g"], f32)
    ln2_b = np.asarray(inputs["ln2_b"], f32)
    w_mlp1 = np.asarray(inputs["w_mlp1"], f32)
    b_mlp1 = np.asarray(inputs["b_mlp1"], f32)
    w_mlp2 = np.asarray(inputs["w_mlp2"], f32)
    b_mlp2 = np.asarray(inputs["b_mlp2"], f32)

    scale = DH ** -0.5
    wqkv_eff = (w_qkv * ln1_g[:, None]).astype(f32)
    bqkv = (ln1_b @ w_qkv).astype(f32)
    wq = wqkv_eff[:, :C] * scale
    wk = wqkv_eff[:, C : 2 * C]
    wvm = wqkv_eff[:, 2 * C :]
    bq = bqkv[:C] * scale
    bk = bqkv[C : 2 * C]
    bv = bqkv[2 * C :]

    wqk = np.empty((128, 2, NP, NJP, 2, 128), dtype=F8)
    for qk, (wm, s) in enumerate(((wq, S_WQ), (wk, S_WK))):
        pk = _pack_pairs(wm, s)  # [128, 4, 2, 1024]
        for p in range(NP):
            wqk[:, qk, p, :, :, :] = pk[:, :, :, p * 128 : (p + 1) * 128]
    wqk_flat = np.ascontiguousarray(wqk.reshape(128, -1))

    wv_flat = np.ascontiguousarray(_pack_pairs(wvm, S_WV).reshape(128, -1))
    wproj_flat = np.ascontiguousarray(_pack_pairs(w_proj, S_WP).reshape(128, -1))

    def pack_with_residual(w, s):
        K2, M = w.shape
        nj = K2 // 256
        ws = np.clip((w * s).astype(np.float32), -240.0, 240.0)
        main = np.empty((128, nj, 2, M), dtype=F8)
        resid = np.empty((128, nj, 2, M), dtype=F8)
        for j in range(nj):
            for t in range(2):
                blk = ws[(2 * j + t) * 128 : (2 * j + t + 1) * 128, :]
                m8 = blk.astype(F8)
                main[:, j, t, :] = m8
                resid[:, j, t, :] = (blk - m8.astype(np.float32)).astype(F8)
        return main, resid

    w1_eff = (w_mlp1 * ln2_g[:, None]).astype(f32)
    w1_flat, w1r_flat = pack_with_residual(w1_eff, S_W1)
    w2_flat, w2r_flat = pack_with_residual(w_mlp2, S_W2)

    bqk_t = np.zeros((128, 16), f32)
    for p in range(NP):
        bqk_t[:, p] = bq[p * 128 : (p + 1) * 128] * S_Q
        bqk_t[:, 8 + p] = bk[p * 128 : (p + 1) * 128] * S_K
    b1_eff = (b_mlp1 + ln2_b @ w_mlp1).astype(f32)
    b1_t = np.ascontiguousarray(b1_eff.reshape(NHB, 128).T) * S_H

    e00 = np.zeros((1, 2, 128), dtype=F8)
    e00[0, 0, :] = 1.0
    bm2_t = np.zeros((1, 2, C), dtype=F8)
    bm2_t[0, 0, :] = np.clip(b_mlp2 * (S_H * S_W2), -240, 240).astype(F8)

    # attention averages (v + bv), so out_attn = y@w_proj + (bv@w_proj + b_proj)
    cvec = (bv @ w_proj + b_proj).astype(f32)

    x_bf = x.astype(BF).reshape(2 * NTT, 128, C)
    xc_bf = (x + cvec[None, :]).astype(BF).reshape(2 * NTT, 128, C)

    in_map = {
        "x": x_bf,
        "xc": xc_bf,
        "wqk": wqk_flat,
        "wv": wv_flat,
        "wproj": wproj_flat,
        "w1": np.ascontiguousarray(w1_flat),
        "w1r": np.ascontiguousarray(w1r_flat),
        "w2": np.ascontiguousarray(w2_flat),
        "w2r": np.ascontiguousarray(w2r_flat),
        "bqk": bqk_t,
        "b1": np.ascontiguousarray(b1_t),
        "e00": np.ascontiguousarray(e00.reshape(1, -1)),
        "bm2": np.ascontiguousarray(bm2_t.reshape(1, -1)),
        "ident": np.eye(128, dtype=BF),
        "ones4": np.full((1, 64), S_Y, f32),
    }
    return [in_map]


def kernel(**inputs):
    nc = _get_nc()
    in_maps = _host_prep(inputs)
    res = run_bass_kernel_spmd(nc, in_maps, core_ids=[0])
    out = res.results[0]["out"]
    return out.reshape(B, N, C).astype(np.float32)
